# revision 1
# baseline (speedup 1.0000x reference)
"""MLA attention kernel for 8 Trainium2 NeuronCores.

Sharding: core i -> batch b = i//4, head group hg = i%4 (32 heads each).
Latent down-projections replicated within a batch group; Wq_up/Wq_rope/
Wk_up/Wv_up/Wo sharded by head.  Host sums the 4 partial outputs per batch.

Device program (identical on all cores, SPMD over different data):
  - all matmuls bf16 with fp32 PSUM accumulation
  - projections computed feature-major (features on partitions) so that
    attention scores S^T[k, q] = kT.T @ qT need no transposes
  - softmax: exp on ScalarE (scale 1/sqrt(96) folded in, no max subtraction:
    scores are ~N(0,1)), denominator via an appended ones-column of V in the
    attn@V matmul, division via DVE reciprocal + gpsimd partition broadcast
"""

import sys

sys.path.insert(0, "/opt/trn_rl_repo")

import numpy as np
import ml_dtypes

import concourse.bass as bass
import concourse.tile as tile
from concourse import bacc, mybir
from concourse.bass_utils import run_bass_kernel_spmd

P = 128
T = 1024          # tokens per batch
DM = 4096         # d_model
KX = DM // P      # 32 feature chunks of x
LAT = 512         # latent dim
LC = LAT // P     # 4 latent chunks
NHC = 32          # heads per core
DH = 32           # head dim (compressed part)
DR = 64           # rope dim per head
NB = 2            # batch
SCALE = 1.0 / float(np.sqrt(DH + DR))

BF = mybir.dt.bfloat16
F32 = mybir.dt.float32

_CACHE = {}


def _build_program():
    nc = bacc.Bacc("TRN2", target_bir_lowering=False, num_devices=8)

    xT = nc.declare_dram_parameter("xT", [DM, T], BF, isOutput=False)
    wqd = nc.declare_dram_parameter("wqd", [DM, LAT], BF, isOutput=False)
    wkvd = nc.declare_dram_parameter("wkvd", [DM, LAT], BF, isOutput=False)
    wqu = nc.declare_dram_parameter("wqu", [LAT, NHC * DH], BF, isOutput=False)
    wku = nc.declare_dram_parameter("wku", [LAT, NHC * DH], BF, isOutput=False)
    wvu = nc.declare_dram_parameter("wvu", [LAT, NHC * DH], BF, isOutput=False)
    wqr = nc.declare_dram_parameter("wqr", [DM, NHC * DR], BF, isOutput=False)
    wkr = nc.declare_dram_parameter("wkr", [DM, DR], BF, isOutput=False)
    wo = nc.declare_dram_parameter("wo", [NHC * DH, DM], BF, isOutput=False)
    out = nc.declare_dram_parameter("out", [T, DM], F32, isOutput=True)

    from contextlib import ExitStack

    with tile.TileContext(nc) as tc, ExitStack() as octx:
        const = octx.enter_context(tc.tile_pool(name="const", bufs=1))

        # Resident tensors (bf16): x transposed, Wk_rope, projections outputs.
        xT_sb = const.tile([P, KX, T], BF, name="xT_sb")
        xT_r = xT[:].rearrange("(ko p) t -> p ko t", p=P)
        wkr_sb = const.tile([P, KX, DR], BF, name="wkr_sb")
        wvu_sb = const.tile([P, LC, NHC * DH], BF, name="wvu_sb")

        cq_sb = const.tile([P, LC, T], BF, name="cq_sb")      # c_q^T
        ckv_sb = const.tile([P, LC, T], BF, name="ckv_sb")    # c_kv^T
        kr_sb = const.tile([DR, T], BF, name="kr_sb")         # k_rope^T (shared)
        # v token-major, per (key-chunk, head): cols 0:32 = v, col 32 = ones
        v_sb = const.tile([P, 8, NHC, 34], BF, name="v_sb")
        # attention output, feature-major: head h -> [32*(h%4):.., h//4, :]
        aout_sb = const.tile([P, 8, T], BF, name="aout_sb")

        nc.vector.memset(v_sb[:, :, :, 32:33], 1.0)

        with ExitStack() as ctx:
            wpool = ctx.enter_context(tc.tile_pool(name="wpool", bufs=3))
            cpp = ctx.enter_context(tc.tile_pool(name="cpp", bufs=3, space="PSUM"))
            qkpool = ctx.enter_context(tc.tile_pool(name="qkpool", bufs=12))
            ppool = ctx.enter_context(tc.tile_pool(name="ppool", bufs=2))
            spp = ctx.enter_context(tc.tile_pool(name="spp", bufs=3, space="PSUM"))
            avp = ctx.enter_context(tc.tile_pool(name="avp", bufs=2, space="PSUM"))
            rpool = ctx.enter_context(tc.tile_pool(name="rpool", bufs=3))
            # (rc/rr on separate tags: a shared tag serializes the per-head
            # reciprocal->broadcast->mul chains and stalls avp slot release)

            # x loaded first (PE starts only once data is in — starting it
            # earlier just runs sparse work at the cold HAM half-clock).
            for kq in range(4):
                nc.sync.dma_start(
                    out=xT_sb[:, kq * 8:(kq + 1) * 8, :],
                    in_=xT_r[:, kq * 8:(kq + 1) * 8, :],
                )
            nc.sync.dma_start(
                out=wkr_sb[:], in_=wkr[:].rearrange("(ko p) d -> p ko d", p=P)
            )
            nc.sync.dma_start(
                out=wvu_sb[:], in_=wvu[:].rearrange("(c p) m -> p c m", p=P)
            )

            # ---- Phase B: latent down-projections (feature-major outputs) ----
            for wd, cdst in ((wqd, cq_sb), (wkvd, ckv_sb)):
                for m in range(LC):
                    wslab = wpool.tile([P, KX, P], BF, tag="wqrs", name="bslab")
                    b_src = wd[:, m * P:(m + 1) * P].rearrange(
                        "(ko p) m -> p ko m", p=P
                    )
                    nc.sync.dma_start(out=wslab[:, 0:16, :], in_=b_src[:, 0:16, :])
                    nc.sync.dma_start(out=wslab[:, 16:KX, :], in_=b_src[:, 16:KX, :])
                    for hf in range(2):
                        ps = cpp.tile([P, 512], F32, tag="cps")
                        for k in range(KX):
                            nc.tensor.matmul(
                                ps[:],
                                wslab[:, k, :],
                                xT_sb[:, k, hf * 512:(hf + 1) * 512],
                                start=(k == 0),
                                stop=(k == KX - 1),
                            )
                        nc.vector.tensor_copy(
                            out=cdst[:, m, hf * 512:(hf + 1) * 512], in_=ps[:]
                        )

            # k_rope^T [64, T]
            for hf in range(2):
                ps = cpp.tile([P, 512], F32, tag="cps")
                for k in range(KX):
                    nc.tensor.matmul(
                        ps[:DR, :],
                        wkr_sb[:, k, :],
                        xT_sb[:, k, hf * 512:(hf + 1) * 512],
                        start=(k == 0),
                        stop=(k == KX - 1),
                    )
                nc.vector.tensor_copy(
                    out=kr_sb[:, hf * 512:(hf + 1) * 512], in_=ps[:DR, :]
                )

            # ---- Phase V: v = c_kv @ Wv_up (token-major), interleaved heads ----
            for tt in range(8):
                for hf in range(2):
                    ps = cpp.tile([P, 512], F32, tag="cps")
                    for lc in range(LC):
                        nc.tensor.matmul(
                            ps[:],
                            ckv_sb[:, lc, tt * P:(tt + 1) * P],
                            wvu_sb[:, lc, hf * 512:(hf + 1) * 512],
                            start=(lc == 0),
                            stop=(lc == LC - 1),
                        )
                    # scatter 16 heads x 32 dims into v_sb[:, tt, h, 0:32]
                    nc.vector.tensor_copy(
                        out=v_sb[:, tt, hf * 16:(hf + 1) * 16, 0:32],
                        in_=ps[:].rearrange("p (h d) -> p h d", h=16),
                    )

            # ---- Phase C+D: project q/k per group of 4 heads, then attend.
            # Projections are emitted one group AHEAD of attention so the PE
            # always has independent work while a group's psum->qkt copies
            # (DVE) and exp (ACT) chains drain.
            def emit_proj(g):
                qt = []
                kt = []
                for j in range(4):
                    qtj = qkpool.tile([P, T], BF, tag="qkt", name=f"qt{g}_{j}")
                    ktj = qkpool.tile([P, T], BF, tag="qkt", name=f"kt{g}_{j}")
                    qt.append(qtj)
                    kt.append(ktj)
                    # shared k_rope rows
                    nc.vector.tensor_copy(out=ktj[0:DR, :], in_=kr_sb[:])

                # q_rope: wqr m-slabs 2g, 2g+1 -> heads (2*ms, 2*ms+1) rows 0:64
                for s in range(2):
                    ms = 2 * g + s
                    wslab = wpool.tile([P, KX, P], BF, tag="wqrs", name="qrslab")
                    qr_src = wqr[:, ms * P:(ms + 1) * P].rearrange(
                        "(ko p) m -> p ko m", p=P
                    )
                    # split: the k-loop starts on the first half while the
                    # second half is still streaming (Tile tracks sub-ranges)
                    nc.sync.dma_start(out=wslab[:, 0:16, :], in_=qr_src[:, 0:16, :])
                    nc.sync.dma_start(out=wslab[:, 16:KX, :], in_=qr_src[:, 16:KX, :])
                    for hf in range(2):
                        ps = cpp.tile([P, 512], F32, tag="cps", name="qr_ps")
                        for k in range(KX):
                            nc.tensor.matmul(
                                ps[:],
                                wslab[:, k, :],
                                xT_sb[:, k, hf * 512:(hf + 1) * 512],
                                start=(k == 0),
                                stop=(k == KX - 1),
                            )
                        sl = slice(hf * 512, (hf + 1) * 512)
                        nc.vector.tensor_copy(out=qt[2 * s][0:DR, sl], in_=ps[0:DR, :])
                        nc.vector.tensor_copy(out=qt[2 * s + 1][0:DR, sl], in_=ps[DR:P, :])

                # q_c / k_c: up-projection slab g (128 cols = 4 heads) rows 64:96
                for wu, dst in ((wqu, qt), (wku, kt)):
                    wslab = wpool.tile([P, LC, P], BF, tag="wups", name="upslab")
                    nc.sync.dma_start(
                        out=wslab[:],
                        in_=wu[:, g * P:(g + 1) * P].rearrange(
                            "(c p) m -> p c m", p=P
                        ),
                    )
                    for hf in range(2):
                        ps = cpp.tile([P, 512], F32, tag="cps", name="up_ps")
                        for lc in range(LC):
                            nc.tensor.matmul(
                                ps[:],
                                wslab[:, lc, :],
                                (cq_sb if wu is wqu else ckv_sb)[
                                    :, lc, hf * 512:(hf + 1) * 512
                                ],
                                start=(lc == 0),
                                stop=(lc == LC - 1),
                            )
                        sl = slice(hf * 512, (hf + 1) * 512)
                        for j in range(4):
                            nc.vector.tensor_copy(
                                out=dst[j][DR:DR + DH, sl],
                                in_=ps[j * DH:(j + 1) * DH, :],
                            )
                return qt, kt

            def emit_attn(g, qt, kt):
                for j in range(4):
                    h = 4 * g + j
                    for qc in range(2):
                        qsl = slice(qc * 512, (qc + 1) * 512)
                        probs = ppool.tile([P, 8, 512], BF, tag="probs", name="probs")
                        for kc in range(8):
                            sp = spp.tile([P, 512], F32, tag="sps", name="sps")
                            nc.tensor.matmul(
                                sp[:],
                                kt[j][0:96, kc * P:(kc + 1) * P],
                                qt[j][0:96, qsl],
                                start=True,
                                stop=True,
                            )
                            nc.scalar.activation(
                                out=probs[:, kc, :],
                                in_=sp[:],
                                func=mybir.ActivationFunctionType.Exp,
                                scale=SCALE,
                            )
                        av = avp.tile([33, 512], F32, tag="avp", name="av")
                        for kc in range(8):
                            nc.tensor.matmul(
                                av[:],
                                v_sb[:, kc, h, 0:33],
                                probs[:, kc, :],
                                start=(kc == 0),
                                stop=(kc == 7),
                            )
                        recip = rpool.tile([1, 512], F32, tag="rc", name="recip")
                        nc.vector.reciprocal(recip[:], av[32:33, :])
                        rrep = rpool.tile([DH, 512], F32, tag="rr", name="rrep")
                        nc.gpsimd.partition_broadcast(rrep[:], recip[:])
                        nc.vector.tensor_mul(
                            out=aout_sb[j * DH:(j + 1) * DH, g, qsl],
                            in0=av[0:DH, :],
                            in1=rrep[:],
                        )

            for g in range(8):
                qt, kt = emit_proj(g)
                emit_attn(g, qt, kt)

        # ---- Phase E: out = aout^T @ Wo  (token-major), Wo streamed once ----
        with ExitStack() as ctx:
            wop = ctx.enter_context(tc.tile_pool(name="wop", bufs=2))
            epp = ctx.enter_context(tc.tile_pool(name="epp", bufs=8, space="PSUM"))
            eop = ctx.enter_context(tc.tile_pool(name="eop", bufs=4))
            for n in range(8):
                woslab = wop.tile([P, 8, 512], BF, tag="wos")
                wo_src = wo[:, n * 512:(n + 1) * 512].rearrange(
                    "(kc p) m -> p kc m", p=P
                )
                nc.sync.dma_start(out=woslab[:, 0:4, :], in_=wo_src[:, 0:4, :])
                nc.sync.dma_start(out=woslab[:, 4:8, :], in_=wo_src[:, 4:8, :])
                pss = [
                    epp.tile([P, 512], F32, tag="eps", name=f"eps_{n}_{i}")
                    for i in range(8)
                ]
                for kc in range(8):
                    for tt in range(8):
                        nc.tensor.matmul(
                            pss[tt][:],
                            aout_sb[:, kc, tt * P:(tt + 1) * P],
                            woslab[:, kc, :],
                            start=(kc == 0),
                            stop=(kc == 7),
                        )
                for tt in range(8):
                    ot = eop.tile([P, 512], F32, tag="eot")
                    nc.any.tensor_copy(out=ot[:], in_=pss[tt][:])
                    nc.sync.dma_start(
                        out=out[tt * P:(tt + 1) * P, n * 512:(n + 1) * 512],
                        in_=ot[:],
                    )

    nc.compile()
    return nc


def _prep_inputs(inputs):
    bf = ml_dtypes.bfloat16
    x = np.asarray(inputs["x"], dtype=np.float32)
    Wq_down = np.asarray(inputs["Wq_down"], dtype=np.float32).astype(bf)
    Wkv_down = np.asarray(inputs["Wkv_down"], dtype=np.float32).astype(bf)
    Wq_up = np.asarray(inputs["Wq_up"], dtype=np.float32).astype(bf)
    Wk_up = np.asarray(inputs["Wk_up"], dtype=np.float32).astype(bf)
    Wv_up = np.asarray(inputs["Wv_up"], dtype=np.float32).astype(bf)
    Wq_rope = np.asarray(inputs["Wq_rope"], dtype=np.float32).astype(bf)
    Wk_rope = np.asarray(inputs["Wk_rope"], dtype=np.float32).astype(bf)
    Wo = np.asarray(inputs["Wo"], dtype=np.float32).astype(bf)

    xT = [np.ascontiguousarray(x[b].T).astype(bf) for b in range(NB)]

    in_maps = []
    for core in range(8):
        b = core // 4
        hg = core % 4
        hs = slice(hg * NHC * DH, (hg + 1) * NHC * DH)        # head-dim cols
        rs = slice(hg * NHC * DR, (hg + 1) * NHC * DR)        # rope cols
        in_maps.append(
            {
                "xT": xT[b],
                "wqd": Wq_down,
                "wkvd": Wkv_down,
                "wqu": np.ascontiguousarray(Wq_up[:, hs]),
                "wku": np.ascontiguousarray(Wk_up[:, hs]),
                "wvu": np.ascontiguousarray(Wv_up[:, hs]),
                "wqr": np.ascontiguousarray(Wq_rope[:, rs]),
                "wkr": Wk_rope,
                "wo": np.ascontiguousarray(Wo[hs, :]),
            }
        )
    return in_maps


def kernel(**inputs):
    if "nc" not in _CACHE:
        _CACHE["nc"] = _build_program()
    nc = _CACHE["nc"]
    in_maps = _prep_inputs(inputs)
    res = run_bass_kernel_spmd(nc, in_maps, list(range(8)))
    out = np.zeros((NB, T, DM), dtype=np.float32)
    for core in range(8):
        out[core // 4] += res.results[core]["out"]
    return out

